# revision 13
# baseline (speedup 1.0000x reference)
"""BKT model (MLP + per-chain 2-state HMM scan) on 8 Trainium2 NeuronCores.

Strategy
--------
Data-parallel over batch: core m handles batch rows [8m, 8m+8).

The reference scans T=1024 steps sequentially, but each of the 500 chains is
visited only ~2x per sequence (max 11).  Host-side we reorganize each core's
8*1024 timesteps by (chain, visit-index): the 4000 (batch,chain) segments are
pooled per core and sorted by visit count descending, so that in "round" r the
active segments are exactly a prefix.  The device then runs:

  Phase A (PE): MLP over the permuted rows: H^T = tanh(W1^T X^T + b1),
                O^T = W2^T H^T + b2, in float32r (TF32-ish, 1 cyc/row).
  Phase B (DVE/ACT): per-visit HMM quantities in probability space
                (sigmoid instead of log-softmax; exact reformulation).
  Phase C: V_max sequential rounds; each round is a fully vectorized
                [128 x c_r] update of all active segments (alpha recurrence +
                normalized output log-probs).  No gathers: all indexing is
                baked into the host-side permutation of the MLP input.

Outputs are scattered back to (b, t) order on the host.
"""

import numpy as np

import concourse.bass as bass
import concourse.tile as tile
import concourse.mybir as mybir
from concourse import bacc
from concourse.bass_utils import run_bass_kernel_spmd
from concourse.masks import make_identity

B, T, NF, NH, NK, NS = 64, 1024, 512, 512, 500, 2
NCORES, BPC, P = 8, 8, 128
F32 = mybir.dt.float32
F32R = mybir.dt.float32r
AF = mybir.ActivationFunctionType
OP = mybir.AluOpType
BF16 = mybir.dt.bfloat16
MM_BF16 = True  # bf16 matmul path: host-converted bf16 inputs, HWDGE loads


# ---------------------------------------------------------------------------
# host-side layout
# ---------------------------------------------------------------------------

def _build_layout(kc):
    kc = np.asarray(kc)
    counts = np.zeros((B, NK), dtype=np.int64)
    for b in range(B):
        np.add.at(counts[b], kc[b].astype(np.int64), 1)
    Vmax = int(counts.max())

    seg_order = []
    n_r = np.zeros((NCORES, Vmax), dtype=np.int64)
    for m in range(NCORES):
        cnt = counts[m * BPC:(m + 1) * BPC].reshape(-1)
        order = np.argsort(-cnt, kind="stable")
        seg_order.append(order)
        for r in range(Vmax):
            n_r[m, r] = int((cnt > r).sum())

    c_r = np.maximum(1, (n_r.max(axis=0) + 127) // 128).astype(np.int64)
    Qc = int(c_r.sum())
    pad = (-Qc) % 4
    c_r[-1] += pad
    Qc += pad
    off_r = np.concatenate([[0], np.cumsum(c_r)[:-1]]).astype(np.int64)
    # chunks: unions of consecutive rounds whose end column is a multiple of 4
    # (so each 512-position matmul tile maps to exactly one chunk)
    chunks = []
    start_r = 0
    for r in range(Vmax):
        end_col = int(off_r[r] + c_r[r])
        if end_col % 4 == 0:
            col0 = int(off_r[start_r])
            chunks.append((start_r, r + 1, col0, end_col - col0))
            start_r = r + 1
    assert start_r == Vmax
    return dict(Vmax=Vmax, c_r=c_r, off_r=off_r, Qc=Qc, Q=128 * Qc,
                seg_order=seg_order, chunks=chunks)


def _build_host_tensors(inputs, lay):
    kc = np.asarray(inputs["kc"]).astype(np.int64)
    corr = np.asarray(inputs["corr"]).astype(np.int64)
    FM = np.ascontiguousarray(np.asarray(inputs["FM"], dtype=np.float32))
    obs = np.asarray(inputs["obs_logits"], dtype=np.float32)
    trans = np.asarray(inputs["trans_logits"], dtype=np.float32)
    init = np.asarray(inputs["init_logits"], dtype=np.float32)

    Vmax, c_r, off_r, Qc, Q = (lay["Vmax"], lay["c_r"], lay["off_r"],
                               lay["Qc"], lay["Q"])
    FMf = FM.reshape(-1, NF)

    per_core = []
    for m in range(NCORES):
        seg = lay["seg_order"][m]
        seg_rank = np.empty(BPC * NK, dtype=np.int64)
        seg_rank[seg] = np.arange(BPC * NK)

        perm = np.zeros(Q, dtype=np.int64)
        valid = np.zeros(Q, dtype=bool)

        for bl in range(BPC):
            b = m * BPC + bl
            ord_t = np.argsort(kc[b], kind="stable")
            ch = kc[b][ord_t]
            visit = np.arange(T) - np.searchsorted(ch, ch)
            s = seg_rank[bl * NK + ch]
            q = (off_r[visit] + s // 128) * 128 + (s % 128)
            perm[q] = b * T + ord_t
            valid[q] = True

        rows = perm
        ch_of_q = kc.reshape(-1)[rows]
        y_of_q = corr.reshape(-1)[rows]

        def plane(vals):
            return np.ascontiguousarray(vals.reshape(Qc, 128).T)

        og = obs[ch_of_q].astype(np.float64)
        tg = trans[ch_of_q].astype(np.float64)
        # obs-logit difference (log-odds of obs=1) per state
        ogdv = og[:, :, 1] - og[:, :, 0]
        ogd = np.concatenate([plane(ogdv[:, 0].astype(np.float32)),
                              plane(ogdv[:, 1].astype(np.float32))], axis=1)
        # exact transition probs P(next=0 | prev=j) (softmax over dim 0)
        t0pv = 1.0 / (1.0 + np.exp(tg[:, 1, :] - tg[:, 0, :]))
        t0p = np.concatenate([plane(t0pv[:, 0].astype(np.float32)),
                              plane(t0pv[:, 1].astype(np.float32))], axis=1)
        t1p = np.concatenate([plane((1.0 - t0pv[:, 0]).astype(np.float32)),
                              plane((1.0 - t0pv[:, 1]).astype(np.float32))],
                             axis=1)
        sgn = plane((2.0 * y_of_q - 1.0).astype(np.float32))

        Sc = 32
        igf = np.zeros((128, 2 * Sc), dtype=np.float32)
        seg_chain = seg % NK
        sl = np.arange(BPC * NK)
        igf[sl % 128, sl // 128] = init[seg_chain, 0]
        igf[sl % 128, Sc + sl // 128] = init[seg_chain, 1]

        xT = np.ascontiguousarray(FMf[rows].T)
        if MM_BF16:
            from ml_dtypes import bfloat16
            xT = xT.astype(bfloat16)

        per_core.append(dict(
            xT=xT,
            og0=np.ascontiguousarray(og0, dtype=np.float32),
            og1=np.ascontiguousarray(og1, dtype=np.float32),
            tg0=np.ascontiguousarray(tg0, dtype=np.float32),
            tg1=np.ascontiguousarray(tg1, dtype=np.float32),
            sgn=np.ascontiguousarray(sgn, dtype=np.float32),
            ig=igf,
            perm=perm, valid=valid,
        ))

    w1 = np.ascontiguousarray(np.asarray(inputs["W1"], np.float32))
    b1r = np.ascontiguousarray(
        np.asarray(inputs["b1"], np.float32).reshape(4, 128).T)
    w2r = np.ascontiguousarray(
        np.asarray(inputs["W2"], np.float32).reshape(4, 128, 2)
        .transpose(1, 0, 2).reshape(128, 8))
    b2 = np.ascontiguousarray(np.asarray(inputs["b2"], np.float32))
    if MM_BF16:
        from ml_dtypes import bfloat16
        w1 = w1.astype(bfloat16)
        w2r = w2r.astype(bfloat16)
    shared = dict(w1=w1, b1r=b1r, w2r=w2r, b2=b2)
    return per_core, shared


# ---------------------------------------------------------------------------
# bass kernel
# ---------------------------------------------------------------------------

def _r2(ap, w2):
    """[128, 2*w] -> [128, 2, w] plane split."""
    return ap.rearrange("p (s w) -> p s w", s=2)


def _kernel_body(ctx, tc, lay, dram, repeat=1):
    nc = tc.nc
    Vmax, c_r, off_r, Qc, Q = (lay["Vmax"], lay["c_r"], lay["off_r"],
                               lay["Qc"], lay["Q"])
    NTILE = Q // 512
    cmax = int(max(c_r))

    singles = ctx.enter_context(tc.tile_pool(name="singles", bufs=1))
    xt_pool = ctx.enter_context(tc.tile_pool(name="xt", bufs=4))
    ht_pool = ctx.enter_context(tc.tile_pool(name="ht", bufs=2))
    sm_pool = ctx.enter_context(tc.tile_pool(name="sm", bufs=3))
    rpool = ctx.enter_context(tc.tile_pool(name="rounds", bufs=2))
    psum = ctx.enter_context(tc.tile_pool(name="psum", bufs=1, space="PSUM"))
    psum2 = ctx.enter_context(tc.tile_pool(name="psum2", bufs=2, space="PSUM"))

    ident = singles.tile([P, P], F32, tag="ident")
    make_identity(nc, ident)

    for _rep in range(repeat):
        _kernel_rep(tc, lay, dram, singles, xt_pool, ht_pool, sm_pool, rpool,
                    psum, psum2, ident)


def _kernel_rep(tc, lay, dram, singles, xt_pool, ht_pool, sm_pool, rpool,
                psum, psum2, ident):
    nc = tc.nc
    Vmax, c_r, off_r, Qc, Q = (lay["Vmax"], lay["c_r"], lay["off_r"],
                               lay["Qc"], lay["Q"])
    NTILE = Q // 512
    cmax = int(max(c_r))
    chunks = lay["chunks"]

    # --- small weights early on the ACT HWDGE ring; xt owns the SP ring ---
    MMDT = BF16 if MM_BF16 else F32R
    w1v = dram["w1"].rearrange("(k p) n -> p k n", p=P)
    w1sb = [singles.tile([P, 512], MMDT, tag=f"w1sb{k}", name=f"w1sb{k}")
            for k in range(4)]
    w2sb = singles.tile([P, 8], MMDT, tag="w2sb")
    for k in range(4):
        nc.scalar.dma_start(out=w1sb[k], in_=w1v[:, k, :])
    nc.scalar.dma_start(out=w2sb, in_=dram["w2r"])
    b1sb = singles.tile([P, 4], F32, tag="b1sb")
    nc.scalar.dma_start(out=b1sb, in_=dram["b1r"])
    b2sb = singles.tile([2, 1], F32, tag="b2sb")
    nc.scalar.dma_start(out=b2sb, in_=dram["b2"])

    og0t = singles.tile([P, 2 * Qc], F32, tag="og0t")
    og1t = singles.tile([P, 2 * Qc], F32, tag="og1t")
    tg0t = singles.tile([P, 2 * Qc], F32, tag="tg0t")
    tg1t = singles.tile([P, 2 * Qc], F32, tag="tg1t")
    sgnt = singles.tile([P, Qc], F32, tag="sgnt")
    igt = singles.tile([P, 64], F32, tag="igt")

    outt = singles.tile([P, 2 * Qc], F32, tag="outt")
    # pw planes: s0,s1 = alpha state (na), s2,s3 = py, s4 = deferred py sum
    pwt = singles.tile([P, 5 * Qc], F32, tag="pwt")
    pw5 = pwt.rearrange("p (s w) -> p s w", s=5)
    out3 = _r2(outt, Qc)
    xTv = dram["xT"].rearrange("(k p) q -> p k q", p=P)

    ocat_ch = [singles.tile([P, 2 * w], F32, tag=f"ocat{ci}", name=f"ocat{ci}")
               for ci, (_, _, _, w) in enumerate(chunks)]
    kpl_ch = [singles.tile([P, 8 * w], F32, tag=f"kpl{ci}", name=f"kpl{ci}")
              for ci, (_, _, _, w) in enumerate(chunks)]
    chunk_of_col = np.zeros(Qc, dtype=np.int64)
    for ci, (_, _, col0, w) in enumerate(chunks):
        chunk_of_col[col0:col0 + w] = ci

    state = dict(prev=None)  # [P, 2, w] AP of the previous round's alpha

    def emit_plane_loads():
        nc.scalar.dma_start(out=og0t, in_=dram["og0"])
        nc.scalar.dma_start(out=og1t, in_=dram["og1"])
        nc.scalar.dma_start(out=tg0t, in_=dram["tg0"])
        nc.scalar.dma_start(out=tg1t, in_=dram["tg1"])
        nc.scalar.dma_start(out=sgnt, in_=dram["sgn"])
        nc.scalar.dma_start(out=igt, in_=dram["ig"])
        # init state: a1 = sigmoid(ig1-ig0) = 0.5 + 0.5*tanh((ig1-ig0)/2)
        ad = sm_pool.tile([P, 32], F32, tag="ad", name="ad")
        nc.vector.tensor_sub(ad, igt[:, 32:64], igt[:, 0:32])
        th = sm_pool.tile([P, 32], F32, tag="th", name="th")
        nc.scalar.activation(out=th, in_=ad, func=AF.Tanh, scale=0.5)
        vinit = singles.tile([P, 64], F32, tag="vinit")
        nc.vector.tensor_scalar(out=vinit[:, 32:64], in0=th,
                                scalar1=0.5, scalar2=0.5,
                                op0=OP.mult, op1=OP.add)
        nc.vector.tensor_scalar(out=vinit[:, 0:32], in0=th,
                                scalar1=-0.5, scalar2=0.5,
                                op0=OP.mult, op1=OP.add)
        state["prev"] = vinit.rearrange("p (j w) -> p j w", j=2)

    def phase_b_and_rounds(ci):
        r0, r1, col0, w = chunks[ci]
        oc = ocat_ch[ci]
        o2c = sm_pool.tile([P, 2 * cmax], F32, tag="o2c",
                           name=f"o2c{ci}")[:, 0:2 * w]
        nc.vector.tensor_scalar_mul(o2c, oc, 2.0)
        ogdc = sm_pool.tile([P, 2 * cmax], F32, tag="ogdc",
                            name=f"ogdc{ci}")[:, 0:2 * w]
        nc.vector.tensor_tensor(out=_r2(ogdc, w),
                                in0=_r2(og1t, Qc)[:, :, col0:col0 + w],
                                in1=_r2(og0t, Qc)[:, :, col0:col0 + w],
                                op=OP.subtract)
        g = sm_pool.tile([P, 6 * cmax], F32, tag="g", name=f"g{ci}")[:, 0:6 * w]
        sg = sm_pool.tile([P, 6 * cmax], F32, tag="sg",
                          name=f"sg{ci}")[:, 0:6 * w]
        nc.vector.tensor_sub(g[:, 2 * w:4 * w], ogdc, o2c)
        nc.vector.tensor_tensor(
            out=_r2(g[:, 0:2 * w], w), in0=_r2(g[:, 2 * w:4 * w], w),
            in1=sgnt[:, col0:col0 + w].unsqueeze(1).broadcast_to([P, 2, w]),
            op=OP.mult)
        nc.vector.tensor_tensor(out=_r2(g[:, 4 * w:6 * w], w),
                                in0=_r2(tg0t, Qc)[:, :, col0:col0 + w],
                                in1=_r2(tg1t, Qc)[:, :, col0:col0 + w],
                                op=OP.subtract)
        # sigmoid(x) = 0.5 + 0.5*tanh(x/2): keep ACT on the tanh table set
        nc.scalar.activation(out=sg, in_=g, func=AF.Tanh, scale=0.5)
        nc.vector.tensor_scalar(out=sg, in0=sg, scalar1=0.5, scalar2=0.5,
                                op0=OP.mult, op1=OP.add)
        # sg = [pe0,pe1 | p01,p11 | T00,T01] (probabilities)
        kt = kpl_ch[ci]
        k4 = kt.rearrange("p (h q w) -> p h q w", h=2, q=4)
        nc.vector.tensor_scalar(out=k4[:, :, 2, :], in0=_r2(sg[:, 2 * w:4 * w], w),
                                scalar1=-1.0, scalar2=1.0,
                                op0=OP.mult, op1=OP.add)
        nc.vector.tensor_copy(out=k4[:, :, 3, :], in_=_r2(sg[:, 2 * w:4 * w], w))
        tcm = sm_pool.tile([P, 2 * cmax], F32, tag="tcm",
                           name=f"tcm{ci}")[:, 0:2 * w]
        nc.vector.tensor_scalar(out=tcm, in0=sg[:, 4 * w:6 * w],
                                scalar1=-1.0, scalar2=1.0,
                                op0=OP.mult, op1=OP.add)
        nc.vector.tensor_tensor(out=k4[:, :, 0, :], in0=_r2(sg[:, 4 * w:6 * w], w),
                                in1=_r2(sg[:, 0:2 * w], w), op=OP.mult)
        nc.vector.tensor_tensor(out=k4[:, :, 1, :], in0=_r2(tcm, w),
                                in1=_r2(sg[:, 0:2 * w], w), op=OP.mult)

        k4v = kt.rearrange("p (j q w) -> p j q w", j=2, q=4)
        for r in range(r0, r1):
            c = int(c_r[r]); off = int(off_r[r]); offl = off - col0
            prev = state["prev"]
            u = rpool.tile([P, 8 * cmax], F32, tag="u", name=f"u{r}")[:, 0:8 * c]
            src = prev[:, :, 0:c].unsqueeze(2).broadcast_to([P, 2, 4, c])
            nc.vector.tensor_tensor(
                out=u.rearrange("p (j q w) -> p j q w", j=2, q=4),
                in0=src, in1=k4v[:, :, :, offl:offl + c], op=OP.mult)
            # one add produces [na0 na1 py0 py1] for this round's columns
            nc.vector.tensor_tensor(
                out=pw5[:, 0:4, off:off + c],
                in0=u[:, 0:4 * c].rearrange("p (q w) -> p q w", q=4),
                in1=u[:, 4 * c:8 * c].rearrange("p (q w) -> p q w", q=4),
                op=OP.add)
            nc.vector.tensor_scalar_max(pw5[:, 0:2, off:off + c],
                                        pw5[:, 0:2, off:off + c], 1e-20)
            state["prev"] = pw5[:, 0:2, off:off + c]

    next_chunk = [0]
    st8_q = []

    def finish_tile(n, st8):
        pt = psum2.tile([P, 8], F32, tag="pt", name=f"pt{n}")
        nc.tensor.transpose(out=pt, in_=st8, identity=ident[0:8, 0:8])
        ci = int(chunk_of_col[4 * n])
        _, _, col0, w = chunks[ci]
        nc.vector.tensor_copy(
            out=_r2(ocat_ch[ci], w)[:, :, 4 * n - col0:4 * n - col0 + 4],
            in_=pt.rearrange("p (s c) -> p s c", s=2))
        while (next_chunk[0] < len(chunks)
               and chunks[next_chunk[0]][2] + chunks[next_chunk[0]][3]
               <= 4 * (n + 1)):
            phase_b_and_rounds(next_chunk[0])
            next_chunk[0] += 1

    for n in range(NTILE):
        if n == 2:
            emit_plane_loads()
        xdma = nc.sync.dma_start
        if n == 0:
            xt0 = [xt_pool.tile([P, 512], MMDT, tag=f"xt0_{k}",
                                name=f"xt0_{k}") for k in range(4)]
            for k in range(4):
                xdma(out=xt0[k], in_=xTv[:, k, 0:512])
            xtk = lambda k: xt0[k]
        else:
            xt = xt_pool.tile([P, 4, 512], MMDT, tag="xt", name=f"xt{n}")
            xdma(out=xt, in_=xTv[:, :, n * 512:(n + 1) * 512])
            xtk = lambda k: xt[:, k, :]
        ht = ht_pool.tile([P, 4, 512], MMDT, tag="ht", name=f"ht{n}")
        for m in range(4):
            ph = psum.tile([P, 512], F32, tag=f"h{m}", name=f"h{m}_{n}")
            for k in range(4):
                nc.tensor.matmul(
                    ph,
                    lhsT=w1sb[k][:, m * 128:(m + 1) * 128],
                    rhs=xtk(k),
                    start=(k == 0), stop=(k == 3))
            nc.scalar.activation(out=ht[:, m, :], in_=ph, func=AF.Tanh,
                                 bias=b1sb[:, m:m + 1], scale=1.0)
        po = psum2.tile([2, 512], F32, tag="po", name=f"po{n}")
        for k in range(4):
            nc.tensor.matmul(po, lhsT=w2sb[:, 2 * k:2 * k + 2],
                             rhs=ht[:, k, :], start=(k == 0), stop=(k == 3))
        ots = sm_pool.tile([2, 512], F32, tag="ots", name=f"ots{n}")
        nc.vector.tensor_scalar(out=ots, in0=po, scalar1=b2sb, scalar2=None,
                                op0=OP.add)
        st8 = sm_pool.tile([8, 128], F32, tag="st8", name=f"st8{n}")
        nc.sync.dma_start(out=st8,
                          in_=ots.rearrange("s (c x) -> s c x", c=4))
        st8_q.append((n, st8))
        if len(st8_q) >= 2:
            finish_tile(*st8_q.pop(0))

    while st8_q:
        finish_tile(*st8_q.pop(0))
    while next_chunk[0] < len(chunks):
        phase_b_and_rounds(next_chunk[0])
        next_chunk[0] += 1

    # deferred py sum, then one Ln pass over [py0|py1|s], out = ln(py) - ln(s)
    nc.vector.tensor_add(pw5[:, 4, :], pw5[:, 2, :], pw5[:, 3, :])
    lnp = singles.tile([P, 3 * Qc], F32, tag="lnp")
    nc.scalar.activation(out=lnp, in_=pwt[:, 2 * Qc:5 * Qc], func=AF.Ln)
    lnp3 = lnp.rearrange("p (s w) -> p s w", s=3)
    nc.vector.tensor_tensor(out=out3, in0=lnp3[:, 0:2, :],
                            in1=lnp3[:, 2:3, :].broadcast_to([P, 2, Qc]),
                            op=OP.subtract)
    nc.sync.dma_start(out=dram["out"], in_=outt)


def _build_nc(lay, repeat=1):
    from contextlib import ExitStack
    nc = bacc.Bacc("TRN2", target_bir_lowering=False, debug=False,
                   num_devices=NCORES)
    Qc, Q = lay["Qc"], lay["Q"]
    dram = {}
    def din(name, shape, dt=F32):
        dram[name] = nc.dram_tensor(name, shape, dt, kind="ExternalInput").ap()
    mmin = BF16 if MM_BF16 else F32R
    din("xT", [NF, Q], mmin)
    din("w1", [NF, NH], mmin)
    din("b1r", [P, 4])
    din("w2r", [P, 8], mmin)
    din("b2", [2])
    din("og0", [P, 2 * Qc])
    din("og1", [P, 2 * Qc])
    din("tg0", [P, 2 * Qc])
    din("tg1", [P, 2 * Qc])
    din("sgn", [P, Qc])
    din("ig", [P, 64])
    dram["out"] = nc.dram_tensor("out", [P, 2 * Qc], F32,
                                 kind="ExternalOutput").ap()
    with tile.TileContext(nc) as tc:
        with ExitStack() as ctx:
            _kernel_body(ctx, tc, lay, dram, repeat=repeat)
    nc.compile()
    return nc


_NC_CACHE = {}


def _get_nc(lay):
    key = tuple(int(x) for x in lay["c_r"])
    if key not in _NC_CACHE:
        _NC_CACHE[key] = _build_nc(lay)
    return _NC_CACHE[key]


# ---------------------------------------------------------------------------
# entry point
# ---------------------------------------------------------------------------

def kernel(corr, kc, FM, W1, b1, W2, b2, trans_logits, obs_logits, init_logits,
           _want_results_only=True, _trace=False):
    inputs = dict(corr=corr, kc=kc, FM=FM, W1=W1, b1=b1, W2=W2, b2=b2,
                  trans_logits=trans_logits, obs_logits=obs_logits,
                  init_logits=init_logits)
    lay = _build_layout(kc)
    nc = _get_nc(lay)
    per_core, shared = _build_host_tensors(inputs, lay)

    in_maps = []
    for m in range(NCORES):
        c = per_core[m]
        in_maps.append(dict(
            xT=c["xT"], w1=shared["w1"], b1r=shared["b1r"], w2r=shared["w2r"],
            b2=shared["b2"], og0=c["og0"], og1=c["og1"], tg0=c["tg0"],
            tg1=c["tg1"], sgn=c["sgn"], ig=c["ig"]))

    res = run_bass_kernel_spmd(nc, in_maps, core_ids=list(range(NCORES)),
                               trace=_trace)

    Qc, Q = lay["Qc"], lay["Q"]
    out = np.zeros((B * T, 2), dtype=np.float32)
    J = np.arange(Q) // 128
    p = np.arange(Q) % 128
    for m in range(NCORES):
        OUT = res.results[m]["out"]
        g = per_core[m]["perm"]; v = per_core[m]["valid"]
        out[g[v], 0] = OUT[p[v], J[v]]
        out[g[v], 1] = OUT[p[v], Qc + J[v]]
    out = out.reshape(B, T, 2)
    if _want_results_only:
        return out
    return out, res



# revision 22
# speedup vs baseline: 1.1178x; 1.1178x over previous
"""BKT model (MLP + per-chain 2-state HMM scan) on 8 Trainium2 NeuronCores.

Strategy
--------
Data-parallel over batch: core m handles batch rows [8m, 8m+8).

The reference scans T=1024 steps sequentially, but each of the 500 chains is
visited only ~2x per sequence (max 11).  Host-side we reorganize each core's
8*1024 timesteps by (chain, visit-index): the 4000 (batch,chain) segments are
pooled per core and sorted by visit count descending, so that in "round" r the
active segments are exactly a prefix.  The device then runs:

  Phase A (PE): MLP over the permuted rows: H^T = tanh(W1^T X^T + b1),
                O^T = W2^T H^T + b2, in float32r (TF32-ish, 1 cyc/row).
  Phase B (DVE/ACT): per-visit HMM quantities in probability space
                (sigmoid instead of log-softmax; exact reformulation).
  Phase C: V_max sequential rounds; each round is a fully vectorized
                [128 x c_r] update of all active segments (alpha recurrence +
                normalized output log-probs).  No gathers: all indexing is
                baked into the host-side permutation of the MLP input.

Outputs are scattered back to (b, t) order on the host.
"""

import numpy as np

import concourse.bass as bass
import concourse.tile as tile
import concourse.mybir as mybir
from concourse import bacc
from concourse.bass_utils import run_bass_kernel_spmd
from concourse.masks import make_identity

B, T, NF, NH, NK, NS = 64, 1024, 512, 512, 500, 2
NCORES, BPC, P = 8, 8, 128
F32 = mybir.dt.float32
F32R = mybir.dt.float32r
AF = mybir.ActivationFunctionType
OP = mybir.AluOpType
BF16 = mybir.dt.bfloat16
MM_BF16 = False  # bf16 MMs measured slower than f32r on HW (FWL/LDW interference)


# ---------------------------------------------------------------------------
# host-side layout
# ---------------------------------------------------------------------------

def _build_layout(kc):
    kc = np.asarray(kc)
    counts = np.zeros((B, NK), dtype=np.int64)
    for b in range(B):
        np.add.at(counts[b], kc[b].astype(np.int64), 1)
    Vmax = int(counts.max())

    seg_order = []
    n_r = np.zeros((NCORES, Vmax), dtype=np.int64)
    for m in range(NCORES):
        cnt = counts[m * BPC:(m + 1) * BPC].reshape(-1)
        order = np.argsort(-cnt, kind="stable")
        seg_order.append(order)
        for r in range(Vmax):
            n_r[m, r] = int((cnt > r).sum())

    c_r = np.maximum(1, (n_r.max(axis=0) + 127) // 128).astype(np.int64)
    Qc = int(c_r.sum())
    pad = (-Qc) % 4
    c_r[-1] += pad
    Qc += pad
    off_r = np.concatenate([[0], np.cumsum(c_r)[:-1]]).astype(np.int64)
    # chunks: unions of consecutive rounds whose end column is a multiple of 4
    # (so each 512-position matmul tile maps to exactly one chunk)
    chunks = []
    start_r = 0
    for r in range(Vmax):
        end_col = int(off_r[r] + c_r[r])
        if end_col % 4 == 0:
            col0 = int(off_r[start_r])
            chunks.append((start_r, r + 1, col0, end_col - col0))
            start_r = r + 1
    assert start_r == Vmax
    return dict(Vmax=Vmax, c_r=c_r, off_r=off_r, Qc=Qc, Q=128 * Qc,
                seg_order=seg_order, chunks=chunks)


def _build_host_tensors(inputs, lay):
    kc = np.asarray(inputs["kc"]).astype(np.int64)
    corr = np.asarray(inputs["corr"]).astype(np.int64)
    FM = np.ascontiguousarray(np.asarray(inputs["FM"], dtype=np.float32))
    obs = np.asarray(inputs["obs_logits"], dtype=np.float32)
    trans = np.asarray(inputs["trans_logits"], dtype=np.float32)
    init = np.asarray(inputs["init_logits"], dtype=np.float32)

    Vmax, c_r, off_r, Qc, Q = (lay["Vmax"], lay["c_r"], lay["off_r"],
                               lay["Qc"], lay["Q"])
    FMf = FM.reshape(-1, NF)

    per_core = []
    for m in range(NCORES):
        seg = lay["seg_order"][m]
        seg_rank = np.empty(BPC * NK, dtype=np.int64)
        seg_rank[seg] = np.arange(BPC * NK)

        perm = np.zeros(Q, dtype=np.int64)
        valid = np.zeros(Q, dtype=bool)

        for bl in range(BPC):
            b = m * BPC + bl
            ord_t = np.argsort(kc[b], kind="stable")
            ch = kc[b][ord_t]
            visit = np.arange(T) - np.searchsorted(ch, ch)
            s = seg_rank[bl * NK + ch]
            q = (off_r[visit] + s // 128) * 128 + (s % 128)
            perm[q] = b * T + ord_t
            valid[q] = True

        rows = perm
        ch_of_q = kc.reshape(-1)[rows]
        y_of_q = corr.reshape(-1)[rows]

        def plane(vals):
            return np.ascontiguousarray(vals.reshape(Qc, 128).T)

        og = obs[ch_of_q].astype(np.float64)
        tg = trans[ch_of_q].astype(np.float64)
        # obs-logit difference (log-odds of obs=1) per state
        ogdv = og[:, :, 1] - og[:, :, 0]
        ogd = np.concatenate([plane(ogdv[:, 0].astype(np.float32)),
                              plane(ogdv[:, 1].astype(np.float32))], axis=1)
        # exact transition probs P(next=0 | prev=j) (softmax over dim 0)
        t0pv = 1.0 / (1.0 + np.exp(tg[:, 1, :] - tg[:, 0, :]))
        t0p = np.concatenate([plane(t0pv[:, 0].astype(np.float32)),
                              plane(t0pv[:, 1].astype(np.float32))], axis=1)
        t1p = np.concatenate([plane((1.0 - t0pv[:, 0]).astype(np.float32)),
                              plane((1.0 - t0pv[:, 1]).astype(np.float32))],
                             axis=1)
        sgn = plane((2.0 * y_of_q - 1.0).astype(np.float32))

        Sc = 32
        igf = np.zeros((128, 2 * Sc), dtype=np.float32)
        seg_chain = seg % NK
        sl = np.arange(BPC * NK)
        igf[sl % 128, sl // 128] = init[seg_chain, 0]
        igf[sl % 128, Sc + sl // 128] = init[seg_chain, 1]

        xT = np.ascontiguousarray(FMf[rows].T)
        if MM_BF16:
            from ml_dtypes import bfloat16
            xT = xT.astype(bfloat16)

        per_core.append(dict(
            xT=xT,
            ogd=np.ascontiguousarray(ogd, dtype=np.float32),
            t0p=np.ascontiguousarray(t0p, dtype=np.float32),
            t1p=np.ascontiguousarray(t1p, dtype=np.float32),
            sgn=np.ascontiguousarray(sgn, dtype=np.float32),
            ig=igf,
            perm=perm, valid=valid,
        ))

    w1 = np.ascontiguousarray(np.asarray(inputs["W1"], np.float32))
    b1r = np.ascontiguousarray(
        np.asarray(inputs["b1"], np.float32).reshape(4, 128).T)
    w2r = np.ascontiguousarray(
        np.asarray(inputs["W2"], np.float32).reshape(4, 128, 2)
        .transpose(1, 0, 2).reshape(128, 8))
    b2 = np.ascontiguousarray(np.asarray(inputs["b2"], np.float32))
    if MM_BF16:
        from ml_dtypes import bfloat16
        w1 = w1.astype(bfloat16)
        w2r = w2r.astype(bfloat16)
    shared = dict(w1=w1, b1r=b1r, w2r=w2r, b2=b2)
    return per_core, shared


# ---------------------------------------------------------------------------
# bass kernel
# ---------------------------------------------------------------------------

def _r2(ap, w2):
    """[128, 2*w] -> [128, 2, w] plane split."""
    return ap.rearrange("p (s w) -> p s w", s=2)


def _kernel_body(ctx, tc, lay, dram, repeat=1):
    nc = tc.nc
    Vmax, c_r, off_r, Qc, Q = (lay["Vmax"], lay["c_r"], lay["off_r"],
                               lay["Qc"], lay["Q"])
    NTILE = Q // 512
    cmax = int(max(c_r))

    singles = ctx.enter_context(tc.tile_pool(name="singles", bufs=1))
    xt_pool = ctx.enter_context(tc.tile_pool(name="xt", bufs=4))
    ht_pool = ctx.enter_context(tc.tile_pool(name="ht", bufs=2))
    sm_pool = ctx.enter_context(tc.tile_pool(name="sm", bufs=3))
    rpool = ctx.enter_context(tc.tile_pool(name="rounds", bufs=2))
    psum = ctx.enter_context(tc.tile_pool(name="psum", bufs=1, space="PSUM"))
    psum2 = ctx.enter_context(tc.tile_pool(name="psum2", bufs=2, space="PSUM"))

    ident = singles.tile([P, P], F32, tag="ident")
    make_identity(nc, ident)

    for _rep in range(repeat):
        _kernel_rep(tc, lay, dram, singles, xt_pool, ht_pool, sm_pool, rpool,
                    psum, psum2, ident)


def _kernel_rep(tc, lay, dram, singles, xt_pool, ht_pool, sm_pool, rpool,
                psum, psum2, ident):
    nc = tc.nc
    Vmax, c_r, off_r, Qc, Q = (lay["Vmax"], lay["c_r"], lay["off_r"],
                               lay["Qc"], lay["Q"])
    NTILE = Q // 512
    cmax = int(max(c_r))
    chunks = lay["chunks"]

    # --- small weights early on the ACT HWDGE ring; xt owns the SP ring ---
    MMDT = BF16 if MM_BF16 else F32R
    w1v = dram["w1"].rearrange("(k p) n -> p k n", p=P)
    w1sb = [singles.tile([P, 512], MMDT, tag=f"w1sb{k}", name=f"w1sb{k}")
            for k in range(4)]
    w2sb = singles.tile([P, 8], MMDT, tag="w2sb")
    for k in range(4):
        nc.scalar.dma_start(out=w1sb[k], in_=w1v[:, k, :])
    nc.scalar.dma_start(out=w2sb, in_=dram["w2r"])
    b1sb = singles.tile([P, 4], F32, tag="b1sb")
    nc.scalar.dma_start(out=b1sb, in_=dram["b1r"])
    b2sb = singles.tile([2, 1], F32, tag="b2sb")
    nc.scalar.dma_start(out=b2sb, in_=dram["b2"])

    ogdt = singles.tile([P, 2 * Qc], F32, tag="ogdt")
    t0pt = singles.tile([P, 2 * Qc], F32, tag="t0pt")
    t1pt = singles.tile([P, 2 * Qc], F32, tag="t1pt")
    sgnt = singles.tile([P, Qc], F32, tag="sgnt")
    igt = singles.tile([P, 64], F32, tag="igt")

    # pw planes: s0,s1 = alpha state (na), s2,s3 = py (linear space)
    pwt = singles.tile([P, 4 * Qc], F32, tag="pwt")
    pw5 = pwt.rearrange("p (s w) -> p s w", s=4)
    xTv = dram["xT"].rearrange("(k p) q -> p k q", p=P)

    ocat_ch = [singles.tile([P, 2 * w], F32, tag=f"ocat{ci}", name=f"ocat{ci}")
               for ci, (_, _, _, w) in enumerate(chunks)]
    kpl_ch = [singles.tile([P, 8 * w], F32, tag=f"kpl{ci}", name=f"kpl{ci}")
              for ci, (_, _, _, w) in enumerate(chunks)]
    chunk_of_col = np.zeros(Qc, dtype=np.int64)
    for ci, (_, _, col0, w) in enumerate(chunks):
        chunk_of_col[col0:col0 + w] = ci

    state = dict(prev=None)  # [P, 2, w] AP of the previous round's alpha

    def emit_plane_loads():
        nc.scalar.dma_start(out=ogdt, in_=dram["ogd"])
        nc.scalar.dma_start(out=t0pt, in_=dram["t0p"])
        nc.scalar.dma_start(out=t1pt, in_=dram["t1p"])
        nc.scalar.dma_start(out=sgnt, in_=dram["sgn"])
        nc.scalar.dma_start(out=igt, in_=dram["ig"])
        # init state: a1 = sigmoid(ig1-ig0) = 0.5 + 0.5*tanh((ig1-ig0)/2)
        ad = sm_pool.tile([P, 32], F32, tag="ad", name="ad")
        nc.vector.tensor_sub(ad, igt[:, 32:64], igt[:, 0:32])
        th = sm_pool.tile([P, 32], F32, tag="th", name="th")
        nc.scalar.activation(out=th, in_=ad, func=AF.Tanh, scale=0.5)
        vinit = singles.tile([P, 64], F32, tag="vinit")
        nc.vector.tensor_scalar(out=vinit[:, 32:64], in0=th,
                                scalar1=0.5, scalar2=0.5,
                                op0=OP.mult, op1=OP.add)
        nc.vector.tensor_scalar(out=vinit[:, 0:32], in0=th,
                                scalar1=-0.5, scalar2=0.5,
                                op0=OP.mult, op1=OP.add)
        state["prev"] = vinit.rearrange("p (j w) -> p j w", j=2)

    def phase_b_and_rounds(ci):
        r0, r1, col0, w = chunks[ci]
        oc = ocat_ch[ci]
        o2c = sm_pool.tile([P, 2 * cmax], F32, tag="o2c",
                           name=f"o2c{ci}")[:, 0:2 * w]
        nc.vector.tensor_scalar_mul(o2c, oc, 2.0)
        g = sm_pool.tile([P, 4 * cmax], F32, tag="g", name=f"g{ci}")[:, 0:4 * w]
        sg = sm_pool.tile([P, 4 * cmax], F32, tag="sg",
                          name=f"sg{ci}")[:, 0:4 * w]
        # g = [pe-logit (2w) | obs1-logit (2w)]
        nc.vector.tensor_tensor(out=_r2(g[:, 2 * w:4 * w], w),
                                in0=_r2(ogdt, Qc)[:, :, col0:col0 + w],
                                in1=_r2(o2c, w), op=OP.subtract)
        nc.vector.tensor_tensor(
            out=_r2(g[:, 0:2 * w], w), in0=_r2(g[:, 2 * w:4 * w], w),
            in1=sgnt[:, col0:col0 + w].unsqueeze(1).broadcast_to([P, 2, w]),
            op=OP.mult)
        # sigmoid(x) = 0.5 + 0.5*tanh(x/2): keep ACT on the tanh table set
        nc.scalar.activation(out=sg, in_=g, func=AF.Tanh, scale=0.5)
        nc.vector.tensor_scalar(out=sg, in0=sg, scalar1=0.5, scalar2=0.5,
                                op0=OP.mult, op1=OP.add)
        # sg = [pe0,pe1 | p01,p11] (probabilities); T planes precomputed on host
        kt = kpl_ch[ci]
        k4 = kt.rearrange("p (h q w) -> p h q w", h=2, q=4)
        nc.vector.tensor_scalar(out=k4[:, :, 2, :], in0=_r2(sg[:, 2 * w:4 * w], w),
                                scalar1=-1.0, scalar2=1.0,
                                op0=OP.mult, op1=OP.add)
        nc.vector.tensor_copy(out=k4[:, :, 3, :], in_=_r2(sg[:, 2 * w:4 * w], w))
        nc.vector.tensor_tensor(out=k4[:, :, 0, :],
                                in0=_r2(t0pt, Qc)[:, :, col0:col0 + w],
                                in1=_r2(sg[:, 0:2 * w], w), op=OP.mult)
        nc.vector.tensor_tensor(out=k4[:, :, 1, :],
                                in0=_r2(t1pt, Qc)[:, :, col0:col0 + w],
                                in1=_r2(sg[:, 0:2 * w], w), op=OP.mult)

        k4v = kt.rearrange("p (j q w) -> p j q w", j=2, q=4)
        for r in range(r0, r1):
            c = int(c_r[r]); off = int(off_r[r]); offl = off - col0
            prev = state["prev"]
            u = rpool.tile([P, 8 * cmax], F32, tag="u", name=f"u{r}")[:, 0:8 * c]
            src = prev[:, :, 0:c].unsqueeze(2).broadcast_to([P, 2, 4, c])
            nc.vector.tensor_tensor(
                out=u.rearrange("p (j q w) -> p j q w", j=2, q=4),
                in0=src, in1=k4v[:, :, :, offl:offl + c], op=OP.mult)
            # one add produces [na0 na1 py0 py1] for this round's columns
            nc.vector.tensor_tensor(
                out=pw5[:, 0:4, off:off + c],
                in0=u[:, 0:4 * c].rearrange("p (q w) -> p q w", q=4),
                in1=u[:, 4 * c:8 * c].rearrange("p (q w) -> p q w", q=4),
                op=OP.add)
            nc.vector.tensor_scalar_max(pw5[:, 0:2, off:off + c],
                                        pw5[:, 0:2, off:off + c], 1e-20)
            state["prev"] = pw5[:, 0:2, off:off + c]

    next_chunk = [0]
    st8_q = []

    def finish_tile(n, st8):
        pt = psum2.tile([P, 8], F32, tag="pt", name=f"pt{n}")
        nc.tensor.transpose(out=pt, in_=st8, identity=ident[0:8, 0:8])
        ci = int(chunk_of_col[4 * n])
        _, _, col0, w = chunks[ci]
        nc.vector.tensor_copy(
            out=_r2(ocat_ch[ci], w)[:, :, 4 * n - col0:4 * n - col0 + 4],
            in_=pt.rearrange("p (s c) -> p s c", s=2))
        while (next_chunk[0] < len(chunks)
               and chunks[next_chunk[0]][2] + chunks[next_chunk[0]][3]
               <= 4 * (n + 1)):
            phase_b_and_rounds(next_chunk[0])
            next_chunk[0] += 1

    for n in range(NTILE):
        if n == 2:
            emit_plane_loads()
        xdma = nc.sync.dma_start
        if n == 0:
            xt0 = [xt_pool.tile([P, 512], MMDT, tag=f"xt0_{k}",
                                name=f"xt0_{k}") for k in range(4)]
            for k in range(4):
                xdma(out=xt0[k], in_=xTv[:, k, 0:512])
            xtk = lambda k: xt0[k]
        else:
            xt = xt_pool.tile([P, 4, 512], MMDT, tag="xt", name=f"xt{n}")
            xdma(out=xt, in_=xTv[:, :, n * 512:(n + 1) * 512])
            xtk = lambda k: xt[:, k, :]
        ht = ht_pool.tile([P, 4, 512], MMDT, tag="ht", name=f"ht{n}")
        for m in range(4):
            ph = psum.tile([P, 512], F32, tag=f"h{m}", name=f"h{m}_{n}")
            for k in range(4):
                nc.tensor.matmul(
                    ph,
                    lhsT=w1sb[k][:, m * 128:(m + 1) * 128],
                    rhs=xtk(k),
                    start=(k == 0), stop=(k == 3))
            nc.scalar.activation(out=ht[:, m, :], in_=ph, func=AF.Tanh,
                                 bias=b1sb[:, m:m + 1], scale=1.0)
        po = psum2.tile([2, 512], F32, tag="po", name=f"po{n}")
        for k in range(4):
            nc.tensor.matmul(po, lhsT=w2sb[:, 2 * k:2 * k + 2],
                             rhs=ht[:, k, :], start=(k == 0), stop=(k == 3))
        ots = sm_pool.tile([2, 512], F32, tag="ots", name=f"ots{n}")
        nc.vector.tensor_scalar(out=ots, in0=po, scalar1=b2sb, scalar2=None,
                                op0=OP.add)
        st8 = sm_pool.tile([8, 128], F32, tag="st8", name=f"st8{n}")
        nc.sync.dma_start(out=st8,
                          in_=ots.rearrange("s (c x) -> s c x", c=4))
        st8_q.append((n, st8))
        if len(st8_q) >= 2:
            finish_tile(*st8_q.pop(0))

    while st8_q:
        finish_tile(*st8_q.pop(0))
    while next_chunk[0] < len(chunks):
        phase_b_and_rounds(next_chunk[0])
        next_chunk[0] += 1

    # ship linear-space py planes; the host does log(py_s) - log(py0+py1)
    nc.sync.dma_start(out=dram["out"], in_=pwt[:, 2 * Qc:4 * Qc])


def _build_nc(lay, repeat=1):
    from contextlib import ExitStack
    nc = bacc.Bacc("TRN2", target_bir_lowering=False, debug=False,
                   num_devices=NCORES)
    Qc, Q = lay["Qc"], lay["Q"]
    dram = {}
    def din(name, shape, dt=F32):
        dram[name] = nc.dram_tensor(name, shape, dt, kind="ExternalInput").ap()
    mmin = BF16 if MM_BF16 else F32R
    din("xT", [NF, Q], mmin)
    din("w1", [NF, NH], mmin)
    din("b1r", [P, 4])
    din("w2r", [P, 8], mmin)
    din("b2", [2])
    din("ogd", [P, 2 * Qc])
    din("t0p", [P, 2 * Qc])
    din("t1p", [P, 2 * Qc])
    din("sgn", [P, Qc])
    din("ig", [P, 64])
    dram["out"] = nc.dram_tensor("out", [P, 2 * Qc], F32,
                                 kind="ExternalOutput").ap()
    with tile.TileContext(nc) as tc:
        with ExitStack() as ctx:
            _kernel_body(ctx, tc, lay, dram, repeat=repeat)
    nc.compile()
    return nc


_NC_CACHE = {}


def _get_nc(lay):
    key = tuple(int(x) for x in lay["c_r"])
    if key not in _NC_CACHE:
        _NC_CACHE[key] = _build_nc(lay)
    return _NC_CACHE[key]


# ---------------------------------------------------------------------------
# entry point
# ---------------------------------------------------------------------------

def kernel(corr, kc, FM, W1, b1, W2, b2, trans_logits, obs_logits, init_logits,
           _want_results_only=True, _trace=False):
    inputs = dict(corr=corr, kc=kc, FM=FM, W1=W1, b1=b1, W2=W2, b2=b2,
                  trans_logits=trans_logits, obs_logits=obs_logits,
                  init_logits=init_logits)
    lay = _build_layout(kc)
    nc = _get_nc(lay)
    per_core, shared = _build_host_tensors(inputs, lay)

    in_maps = []
    for m in range(NCORES):
        c = per_core[m]
        in_maps.append(dict(
            xT=c["xT"], w1=shared["w1"], b1r=shared["b1r"], w2r=shared["w2r"],
            b2=shared["b2"], ogd=c["ogd"], t0p=c["t0p"], t1p=c["t1p"],
            sgn=c["sgn"], ig=c["ig"]))

    res = run_bass_kernel_spmd(nc, in_maps, core_ids=list(range(NCORES)),
                               trace=_trace)

    out = np.zeros((B * T, 2), dtype=np.float32)
    for m in range(NCORES):
        scatter_core_output(lay, per_core[m], res.results[m]["out"], out)
    out = out.reshape(B, T, 2)
    if _want_results_only:
        return out
    return out, res


def scatter_core_output(lay, core, OUT, out):
    """OUT: [P, 2*Qc] linear-space py planes -> log-prob rows of `out`."""
    Qc, Q = lay["Qc"], lay["Q"]
    J = np.arange(Q) // 128
    p = np.arange(Q) % 128
    g = core["perm"]; v = core["valid"]
    py0 = OUT[p[v], J[v]].astype(np.float64)
    py1 = OUT[p[v], Qc + J[v]].astype(np.float64)
    s = np.log(py0 + py1)
    out[g[v], 0] = np.log(py0) - s
    out[g[v], 1] = np.log(py1) - s



# revision 23
# speedup vs baseline: 1.1732x; 1.0495x over previous
"""BKT model (MLP + per-chain 2-state HMM scan) on 8 Trainium2 NeuronCores.

Strategy
--------
Data-parallel over batch: core m handles batch rows [8m, 8m+8).

The reference scans T=1024 steps sequentially, but each of the 500 chains is
visited only ~2x per sequence (max 11).  Host-side we reorganize each core's
8*1024 timesteps by (chain, visit-index): the 4000 (batch,chain) segments are
pooled per core and sorted by visit count descending, so that in "round" r the
active segments are exactly a prefix.  The device then runs:

  Phase A (PE): MLP over the permuted rows: H^T = tanh(W1^T X^T + b1),
                O^T = W2^T H^T + b2, in float32r (TF32-ish, 1 cyc/row).
  Phase B (DVE/ACT): per-visit HMM quantities in probability space
                (sigmoid instead of log-softmax; exact reformulation).
  Phase C: V_max sequential rounds; each round is a fully vectorized
                [128 x c_r] update of all active segments (alpha recurrence +
                normalized output log-probs).  No gathers: all indexing is
                baked into the host-side permutation of the MLP input.

Outputs are scattered back to (b, t) order on the host.
"""

import numpy as np

import concourse.bass as bass
import concourse.tile as tile
import concourse.mybir as mybir
from concourse import bacc
from concourse.bass_utils import run_bass_kernel_spmd
from concourse.masks import make_identity

B, T, NF, NH, NK, NS = 64, 1024, 512, 512, 500, 2
NCORES, BPC, P = 8, 8, 128
F32 = mybir.dt.float32
F32R = mybir.dt.float32r
AF = mybir.ActivationFunctionType
OP = mybir.AluOpType
BF16 = mybir.dt.bfloat16
MM_BF16 = False  # bf16 MMs measured slower than f32r on HW (FWL/LDW interference)


# ---------------------------------------------------------------------------
# host-side layout
# ---------------------------------------------------------------------------

def _build_layout(kc):
    kc = np.asarray(kc)
    counts = np.zeros((B, NK), dtype=np.int64)
    for b in range(B):
        np.add.at(counts[b], kc[b].astype(np.int64), 1)
    Vmax = int(counts.max())

    seg_order = []
    n_r = np.zeros((NCORES, Vmax), dtype=np.int64)
    for m in range(NCORES):
        cnt = counts[m * BPC:(m + 1) * BPC].reshape(-1)
        order = np.argsort(-cnt, kind="stable")
        seg_order.append(order)
        for r in range(Vmax):
            n_r[m, r] = int((cnt > r).sum())

    c_r = np.maximum(1, (n_r.max(axis=0) + 127) // 128).astype(np.int64)
    Qc = int(c_r.sum())
    pad = (-Qc) % 4
    c_r[-1] += pad
    Qc += pad
    off_r = np.concatenate([[0], np.cumsum(c_r)[:-1]]).astype(np.int64)
    # chunks: unions of consecutive rounds whose end column is a multiple of 4
    # (so each 512-position matmul tile maps to exactly one chunk)
    chunks = []
    start_r = 0
    for r in range(Vmax):
        end_col = int(off_r[r] + c_r[r])
        if end_col % 4 == 0:
            col0 = int(off_r[start_r])
            chunks.append((start_r, r + 1, col0, end_col - col0))
            start_r = r + 1
    assert start_r == Vmax
    return dict(Vmax=Vmax, c_r=c_r, off_r=off_r, Qc=Qc, Q=128 * Qc,
                seg_order=seg_order, chunks=chunks)


def _build_host_tensors(inputs, lay):
    kc = np.asarray(inputs["kc"]).astype(np.int64)
    corr = np.asarray(inputs["corr"]).astype(np.int64)
    FM = np.ascontiguousarray(np.asarray(inputs["FM"], dtype=np.float32))
    obs = np.asarray(inputs["obs_logits"], dtype=np.float32)
    trans = np.asarray(inputs["trans_logits"], dtype=np.float32)
    init = np.asarray(inputs["init_logits"], dtype=np.float32)

    Vmax, c_r, off_r, Qc, Q = (lay["Vmax"], lay["c_r"], lay["off_r"],
                               lay["Qc"], lay["Q"])
    FMf = FM.reshape(-1, NF)

    per_core = []
    for m in range(NCORES):
        seg = lay["seg_order"][m]
        seg_rank = np.empty(BPC * NK, dtype=np.int64)
        seg_rank[seg] = np.arange(BPC * NK)

        perm = np.zeros(Q, dtype=np.int64)
        valid = np.zeros(Q, dtype=bool)

        for bl in range(BPC):
            b = m * BPC + bl
            ord_t = np.argsort(kc[b], kind="stable")
            ch = kc[b][ord_t]
            visit = np.arange(T) - np.searchsorted(ch, ch)
            s = seg_rank[bl * NK + ch]
            q = (off_r[visit] + s // 128) * 128 + (s % 128)
            perm[q] = b * T + ord_t
            valid[q] = True

        rows = perm
        ch_of_q = kc.reshape(-1)[rows]
        y_of_q = corr.reshape(-1)[rows]

        def plane(vals):
            return np.ascontiguousarray(vals.reshape(Qc, 128).T)

        og = obs[ch_of_q].astype(np.float64)
        tg = trans[ch_of_q].astype(np.float64)
        # obs-logit difference (log-odds of obs=1) per state
        ogdv = og[:, :, 1] - og[:, :, 0]
        ogd = np.concatenate([plane(ogdv[:, 0].astype(np.float32)),
                              plane(ogdv[:, 1].astype(np.float32))], axis=1)
        # exact transition probs P(next=0 | prev=j) (softmax over dim 0)
        t0pv = 1.0 / (1.0 + np.exp(tg[:, 1, :] - tg[:, 0, :]))
        t0p = np.concatenate([plane(t0pv[:, 0].astype(np.float32)),
                              plane(t0pv[:, 1].astype(np.float32))], axis=1)
        t1p = np.concatenate([plane((1.0 - t0pv[:, 0]).astype(np.float32)),
                              plane((1.0 - t0pv[:, 1]).astype(np.float32))],
                             axis=1)
        sgn = plane((2.0 * y_of_q - 1.0).astype(np.float32))

        Sc = 32
        igf = np.zeros((128, 2 * Sc), dtype=np.float32)
        seg_chain = seg % NK
        sl = np.arange(BPC * NK)
        igf[sl % 128, sl // 128] = init[seg_chain, 0]
        igf[sl % 128, Sc + sl // 128] = init[seg_chain, 1]

        xT = np.ascontiguousarray(FMf[rows].T)
        if MM_BF16:
            from ml_dtypes import bfloat16
            xT = xT.astype(bfloat16)

        per_core.append(dict(
            xT=xT,
            ogd=np.ascontiguousarray(ogd, dtype=np.float32),
            t0p=np.ascontiguousarray(t0p, dtype=np.float32),
            t1p=np.ascontiguousarray(t1p, dtype=np.float32),
            sgn=np.ascontiguousarray(sgn, dtype=np.float32),
            ig=igf,
            perm=perm, valid=valid,
        ))

    w1 = np.ascontiguousarray(np.asarray(inputs["W1"], np.float32))
    b1r = np.ascontiguousarray(
        np.asarray(inputs["b1"], np.float32).reshape(4, 128).T)
    w2r = np.ascontiguousarray(
        np.asarray(inputs["W2"], np.float32).reshape(4, 128, 2)
        .transpose(1, 0, 2).reshape(128, 8))
    b2 = np.ascontiguousarray(np.asarray(inputs["b2"], np.float32))
    if MM_BF16:
        from ml_dtypes import bfloat16
        w1 = w1.astype(bfloat16)
        w2r = w2r.astype(bfloat16)
    shared = dict(w1=w1, b1r=b1r, w2r=w2r, b2=b2)
    return per_core, shared


# ---------------------------------------------------------------------------
# bass kernel
# ---------------------------------------------------------------------------

def _r2(ap, w2):
    """[128, 2*w] -> [128, 2, w] plane split."""
    return ap.rearrange("p (s w) -> p s w", s=2)


def _kernel_body(ctx, tc, lay, dram, repeat=1):
    nc = tc.nc
    Vmax, c_r, off_r, Qc, Q = (lay["Vmax"], lay["c_r"], lay["off_r"],
                               lay["Qc"], lay["Q"])
    NTILE = Q // 512
    cmax = int(max(c_r))

    singles = ctx.enter_context(tc.tile_pool(name="singles", bufs=1))
    xt_pool = ctx.enter_context(tc.tile_pool(name="xt", bufs=4))
    ht_pool = ctx.enter_context(tc.tile_pool(name="ht", bufs=2))
    sm_pool = ctx.enter_context(tc.tile_pool(name="sm", bufs=3))
    rpool = ctx.enter_context(tc.tile_pool(name="rounds", bufs=2))
    psum = ctx.enter_context(tc.tile_pool(name="psum", bufs=1, space="PSUM"))
    psum2 = ctx.enter_context(tc.tile_pool(name="psum2", bufs=2, space="PSUM"))

    ident = singles.tile([P, P], F32, tag="ident")
    make_identity(nc, ident)

    for _rep in range(repeat):
        _kernel_rep(tc, lay, dram, singles, xt_pool, ht_pool, sm_pool, rpool,
                    psum, psum2, ident)


def _kernel_rep(tc, lay, dram, singles, xt_pool, ht_pool, sm_pool, rpool,
                psum, psum2, ident):
    nc = tc.nc
    Vmax, c_r, off_r, Qc, Q = (lay["Vmax"], lay["c_r"], lay["off_r"],
                               lay["Qc"], lay["Q"])
    NTILE = Q // 512
    cmax = int(max(c_r))
    chunks = lay["chunks"]

    # --- small weights early on the ACT HWDGE ring; xt owns the SP ring ---
    MMDT = BF16 if MM_BF16 else F32R
    w1v = dram["w1"].rearrange("(k p) n -> p k n", p=P)
    w1sb = [singles.tile([P, 512], MMDT, tag=f"w1sb{k}", name=f"w1sb{k}")
            for k in range(4)]
    w2sb = singles.tile([P, 8], MMDT, tag="w2sb")
    for k in range(4):
        nc.scalar.dma_start(out=w1sb[k], in_=w1v[:, k, :])
    nc.scalar.dma_start(out=w2sb, in_=dram["w2r"])
    b1sb = singles.tile([P, 4], F32, tag="b1sb")
    nc.scalar.dma_start(out=b1sb, in_=dram["b1r"])
    b2sb = singles.tile([2, 1], F32, tag="b2sb")
    nc.scalar.dma_start(out=b2sb, in_=dram["b2"])

    ogdt = singles.tile([P, 2 * Qc], F32, tag="ogdt")
    t0pt = singles.tile([P, 2 * Qc], F32, tag="t0pt")
    t1pt = singles.tile([P, 2 * Qc], F32, tag="t1pt")
    sgnt = singles.tile([P, Qc], F32, tag="sgnt")
    igt = singles.tile([P, 64], F32, tag="igt")

    # pw planes: s0,s1 = alpha state (na), s2,s3 = py (linear space)
    pwt = singles.tile([P, 4 * Qc], F32, tag="pwt")
    pw5 = pwt.rearrange("p (s w) -> p s w", s=4)
    xTv = dram["xT"].rearrange("(k p) q -> p k q", p=P)

    ocat_ch = [singles.tile([P, 2 * w], F32, tag=f"ocat{ci}", name=f"ocat{ci}")
               for ci, (_, _, _, w) in enumerate(chunks)]
    kpl_ch = [singles.tile([P, 8 * w], F32, tag=f"kpl{ci}", name=f"kpl{ci}")
              for ci, (_, _, _, w) in enumerate(chunks)]
    chunk_of_col = np.zeros(Qc, dtype=np.int64)
    for ci, (_, _, col0, w) in enumerate(chunks):
        chunk_of_col[col0:col0 + w] = ci

    state = dict(prev=None)  # [P, 2, w] AP of the previous round's alpha

    def emit_plane_loads():
        nc.scalar.dma_start(out=ogdt, in_=dram["ogd"])
        nc.scalar.dma_start(out=t0pt, in_=dram["t0p"])
        nc.scalar.dma_start(out=t1pt, in_=dram["t1p"])
        nc.scalar.dma_start(out=sgnt, in_=dram["sgn"])
        nc.scalar.dma_start(out=igt, in_=dram["ig"])
        # init state: a1 = sigmoid(ig1-ig0) = 0.5 + 0.5*tanh((ig1-ig0)/2)
        ad = sm_pool.tile([P, 32], F32, tag="ad", name="ad")
        nc.vector.tensor_sub(ad, igt[:, 32:64], igt[:, 0:32])
        th = sm_pool.tile([P, 32], F32, tag="th", name="th")
        nc.scalar.activation(out=th, in_=ad, func=AF.Tanh, scale=0.5)
        vinit = singles.tile([P, 64], F32, tag="vinit")
        nc.vector.tensor_scalar(out=vinit[:, 32:64], in0=th,
                                scalar1=0.5, scalar2=0.5,
                                op0=OP.mult, op1=OP.add)
        nc.vector.tensor_scalar(out=vinit[:, 0:32], in0=th,
                                scalar1=-0.5, scalar2=0.5,
                                op0=OP.mult, op1=OP.add)
        state["prev"] = vinit.rearrange("p (j w) -> p j w", j=2)

    def phase_b_and_rounds(ci):
        r0, r1, col0, w = chunks[ci]
        oc = ocat_ch[ci]
        o2c = sm_pool.tile([P, 2 * cmax], F32, tag="o2c",
                           name=f"o2c{ci}")[:, 0:2 * w]
        nc.vector.tensor_scalar_mul(o2c, oc, 2.0)
        g = sm_pool.tile([P, 4 * cmax], F32, tag="g", name=f"g{ci}")[:, 0:4 * w]
        sg = sm_pool.tile([P, 4 * cmax], F32, tag="sg",
                          name=f"sg{ci}")[:, 0:4 * w]
        # g = [pe-logit (2w) | obs1-logit (2w)]
        nc.vector.tensor_tensor(out=_r2(g[:, 2 * w:4 * w], w),
                                in0=_r2(ogdt, Qc)[:, :, col0:col0 + w],
                                in1=_r2(o2c, w), op=OP.subtract)
        nc.vector.tensor_tensor(
            out=_r2(g[:, 0:2 * w], w), in0=_r2(g[:, 2 * w:4 * w], w),
            in1=sgnt[:, col0:col0 + w].unsqueeze(1).broadcast_to([P, 2, w]),
            op=OP.mult)
        # sigmoid(x) = 0.5 + 0.5*tanh(x/2): keep ACT on the tanh table set
        nc.scalar.activation(out=sg, in_=g, func=AF.Tanh, scale=0.5)
        nc.vector.tensor_scalar(out=sg, in0=sg, scalar1=0.5, scalar2=0.5,
                                op0=OP.mult, op1=OP.add)
        # sg = [pe0,pe1 | p01,p11] (probabilities); T planes precomputed on host
        kt = kpl_ch[ci]
        k4 = kt.rearrange("p (h q w) -> p h q w", h=2, q=4)
        nc.vector.tensor_scalar(out=k4[:, :, 2, :], in0=_r2(sg[:, 2 * w:4 * w], w),
                                scalar1=-1.0, scalar2=1.0,
                                op0=OP.mult, op1=OP.add)
        nc.vector.tensor_copy(out=k4[:, :, 3, :], in_=_r2(sg[:, 2 * w:4 * w], w))
        nc.vector.tensor_tensor(out=k4[:, :, 0, :],
                                in0=_r2(t0pt, Qc)[:, :, col0:col0 + w],
                                in1=_r2(sg[:, 0:2 * w], w), op=OP.mult)
        nc.vector.tensor_tensor(out=k4[:, :, 1, :],
                                in0=_r2(t1pt, Qc)[:, :, col0:col0 + w],
                                in1=_r2(sg[:, 0:2 * w], w), op=OP.mult)

        k4v = kt.rearrange("p (j q w) -> p j q w", j=2, q=4)
        for r in range(r0, r1):
            c = int(c_r[r]); off = int(off_r[r]); offl = off - col0
            prev = state["prev"]
            u = rpool.tile([P, 8 * cmax], F32, tag="u", name=f"u{r}")[:, 0:8 * c]
            src = prev[:, :, 0:c].unsqueeze(2).broadcast_to([P, 2, 4, c])
            nc.vector.tensor_tensor(
                out=u.rearrange("p (j q w) -> p j q w", j=2, q=4),
                in0=src, in1=k4v[:, :, :, offl:offl + c], op=OP.mult)
            # one add produces [na0 na1 py0 py1] for this round's columns
            nc.vector.tensor_tensor(
                out=pw5[:, 0:4, off:off + c],
                in0=u[:, 0:4 * c].rearrange("p (q w) -> p q w", q=4),
                in1=u[:, 4 * c:8 * c].rearrange("p (q w) -> p q w", q=4),
                op=OP.add)
            if c > 2:
                nc.vector.tensor_scalar_max(pw5[:, 0:2, off:off + c],
                                            pw5[:, 0:2, off:off + c], 1e-20)
            state["prev"] = pw5[:, 0:2, off:off + c]

    next_chunk = [0]
    GRP = 4  # tiles per batched PE transpose

    def finish_group(g, st32, ntl):
        # one transpose covers ntl tiles (8 rows each)
        pt = psum2.tile([P, 8 * GRP], F32, tag="pt", name=f"pt{g}")[:, 0:8 * ntl]
        nc.tensor.transpose(out=pt, in_=st32[0:8 * ntl, :],
                            identity=ident[0:8 * ntl, 0:8 * ntl])
        for tl in range(ntl):
            n = g * GRP + tl
            ci = int(chunk_of_col[4 * n])
            _, _, col0, w = chunks[ci]
            nc.vector.tensor_copy(
                out=_r2(ocat_ch[ci], w)[:, :, 4 * n - col0:4 * n - col0 + 4],
                in_=pt[:, 8 * tl:8 * tl + 8].rearrange("p (s c) -> p s c", s=2))
        n_done = g * GRP + ntl
        while (next_chunk[0] < len(chunks)
               and chunks[next_chunk[0]][2] + chunks[next_chunk[0]][3]
               <= 4 * n_done):
            phase_b_and_rounds(next_chunk[0])
            next_chunk[0] += 1

    grp_q = []
    st32 = None
    for n in range(NTILE):
        if n == 2:
            emit_plane_loads()
        xdma = nc.sync.dma_start
        if n == 0:
            xt0 = [xt_pool.tile([P, 512], MMDT, tag=f"xt0_{k}",
                                name=f"xt0_{k}") for k in range(4)]
            for k in range(4):
                xdma(out=xt0[k], in_=xTv[:, k, 0:512])
            xtk = lambda k: xt0[k]
        else:
            xt = xt_pool.tile([P, 4, 512], MMDT, tag="xt", name=f"xt{n}")
            xdma(out=xt, in_=xTv[:, :, n * 512:(n + 1) * 512])
            xtk = lambda k: xt[:, k, :]
        ht = ht_pool.tile([P, 4, 512], MMDT, tag="ht", name=f"ht{n}")
        for m in range(4):
            ph = psum.tile([P, 512], F32, tag=f"h{m}", name=f"h{m}_{n}")
            for k in range(4):
                nc.tensor.matmul(
                    ph,
                    lhsT=w1sb[k][:, m * 128:(m + 1) * 128],
                    rhs=xtk(k),
                    start=(k == 0), stop=(k == 3))
            nc.scalar.activation(out=ht[:, m, :], in_=ph, func=AF.Tanh,
                                 bias=b1sb[:, m:m + 1], scale=1.0)
        po = psum2.tile([2, 512], F32, tag="po", name=f"po{n}")
        for k in range(4):
            nc.tensor.matmul(po, lhsT=w2sb[:, 2 * k:2 * k + 2],
                             rhs=ht[:, k, :], start=(k == 0), stop=(k == 3))
        ots = sm_pool.tile([2, 512], F32, tag="ots", name=f"ots{n}")
        nc.vector.tensor_scalar(out=ots, in0=po, scalar1=b2sb, scalar2=None,
                                op0=OP.add)
        tl = n % GRP
        if tl == 0:
            st32 = sm_pool.tile([8 * GRP, 128], F32, tag="st32",
                                name=f"st32_{n // GRP}")
        nc.scalar.dma_start(out=st32[8 * tl:8 * tl + 8, :],
                            in_=ots.rearrange("s (c x) -> s c x", c=4))
        if tl == GRP - 1 or n == NTILE - 1:
            grp_q.append((n // GRP, st32, tl + 1))
            if len(grp_q) >= 2:
                finish_group(*grp_q.pop(0))

    while grp_q:
        finish_group(*grp_q.pop(0))
    while next_chunk[0] < len(chunks):
        phase_b_and_rounds(next_chunk[0])
        next_chunk[0] += 1

    # ship linear-space py planes; the host does log(py_s) - log(py0+py1)
    nc.scalar.dma_start(out=dram["out"], in_=pwt[:, 2 * Qc:4 * Qc])


def _build_nc(lay, repeat=1):
    from contextlib import ExitStack
    nc = bacc.Bacc("TRN2", target_bir_lowering=False, debug=False,
                   num_devices=NCORES)
    Qc, Q = lay["Qc"], lay["Q"]
    dram = {}
    def din(name, shape, dt=F32):
        dram[name] = nc.dram_tensor(name, shape, dt, kind="ExternalInput").ap()
    mmin = BF16 if MM_BF16 else F32R
    din("xT", [NF, Q], mmin)
    din("w1", [NF, NH], mmin)
    din("b1r", [P, 4])
    din("w2r", [P, 8], mmin)
    din("b2", [2])
    din("ogd", [P, 2 * Qc])
    din("t0p", [P, 2 * Qc])
    din("t1p", [P, 2 * Qc])
    din("sgn", [P, Qc])
    din("ig", [P, 64])
    dram["out"] = nc.dram_tensor("out", [P, 2 * Qc], F32,
                                 kind="ExternalOutput").ap()
    with tile.TileContext(nc) as tc:
        with ExitStack() as ctx:
            _kernel_body(ctx, tc, lay, dram, repeat=repeat)
    nc.compile()
    return nc


_NC_CACHE = {}


def _get_nc(lay):
    key = tuple(int(x) for x in lay["c_r"])
    if key not in _NC_CACHE:
        _NC_CACHE[key] = _build_nc(lay)
    return _NC_CACHE[key]


# ---------------------------------------------------------------------------
# entry point
# ---------------------------------------------------------------------------

def kernel(corr, kc, FM, W1, b1, W2, b2, trans_logits, obs_logits, init_logits,
           _want_results_only=True, _trace=False):
    inputs = dict(corr=corr, kc=kc, FM=FM, W1=W1, b1=b1, W2=W2, b2=b2,
                  trans_logits=trans_logits, obs_logits=obs_logits,
                  init_logits=init_logits)
    lay = _build_layout(kc)
    nc = _get_nc(lay)
    per_core, shared = _build_host_tensors(inputs, lay)

    in_maps = []
    for m in range(NCORES):
        c = per_core[m]
        in_maps.append(dict(
            xT=c["xT"], w1=shared["w1"], b1r=shared["b1r"], w2r=shared["w2r"],
            b2=shared["b2"], ogd=c["ogd"], t0p=c["t0p"], t1p=c["t1p"],
            sgn=c["sgn"], ig=c["ig"]))

    res = run_bass_kernel_spmd(nc, in_maps, core_ids=list(range(NCORES)),
                               trace=_trace)

    out = np.zeros((B * T, 2), dtype=np.float32)
    for m in range(NCORES):
        scatter_core_output(lay, per_core[m], res.results[m]["out"], out)
    out = out.reshape(B, T, 2)
    if _want_results_only:
        return out
    return out, res


def scatter_core_output(lay, core, OUT, out):
    """OUT: [P, 2*Qc] linear-space py planes -> log-prob rows of `out`."""
    Qc, Q = lay["Qc"], lay["Q"]
    J = np.arange(Q) // 128
    p = np.arange(Q) % 128
    g = core["perm"]; v = core["valid"]
    py0 = OUT[p[v], J[v]].astype(np.float64)
    py1 = OUT[p[v], Qc + J[v]].astype(np.float64)
    s = np.log(py0 + py1)
    out[g[v], 0] = np.log(py0) - s
    out[g[v], 1] = np.log(py1) - s



# revision 27
# speedup vs baseline: 1.1885x; 1.0131x over previous
"""BKT model (MLP + per-chain 2-state HMM scan) on 8 Trainium2 NeuronCores.

Strategy
--------
Data-parallel over batch: core m handles batch rows [8m, 8m+8).

The reference scans T=1024 steps sequentially, but each of the 500 chains is
visited only ~2x per sequence (max 11).  Host-side we reorganize each core's
8*1024 timesteps by (chain, visit-index): the 4000 (batch,chain) segments are
pooled per core and sorted by visit count descending, so that in "round" r the
active segments are exactly a prefix.  The device then runs:

  Phase A (PE): MLP over the permuted rows: H^T = tanh(W1^T X^T + b1),
                O^T = W2^T H^T + b2, in float32r (TF32-ish, 1 cyc/row).
  Phase B (DVE/ACT): per-visit HMM quantities in probability space
                (sigmoid instead of log-softmax; exact reformulation).
  Phase C: V_max sequential rounds; each round is a fully vectorized
                [128 x c_r] update of all active segments (alpha recurrence +
                normalized output log-probs).  No gathers: all indexing is
                baked into the host-side permutation of the MLP input.

Outputs are scattered back to (b, t) order on the host.
"""

import numpy as np

import concourse.bass as bass
import concourse.tile as tile
import concourse.mybir as mybir
from concourse import bacc
from concourse.bass_utils import run_bass_kernel_spmd
from concourse.masks import make_identity

B, T, NF, NH, NK, NS = 64, 1024, 512, 512, 500, 2
NCORES, BPC, P = 8, 8, 128
F32 = mybir.dt.float32
F32R = mybir.dt.float32r
AF = mybir.ActivationFunctionType
OP = mybir.AluOpType
BF16 = mybir.dt.bfloat16
MM_BF16 = False  # bf16 MMs measured slower than f32r on HW (FWL/LDW interference)


# ---------------------------------------------------------------------------
# host-side layout
# ---------------------------------------------------------------------------

def _build_layout(kc):
    kc = np.asarray(kc)
    counts = np.zeros((B, NK), dtype=np.int64)
    for b in range(B):
        np.add.at(counts[b], kc[b].astype(np.int64), 1)
    Vmax = int(counts.max())

    seg_order = []
    n_r = np.zeros((NCORES, Vmax), dtype=np.int64)
    for m in range(NCORES):
        cnt = counts[m * BPC:(m + 1) * BPC].reshape(-1)
        order = np.argsort(-cnt, kind="stable")
        seg_order.append(order)
        for r in range(Vmax):
            n_r[m, r] = int((cnt > r).sum())

    N_r = n_r.max(axis=0).astype(np.int64)  # valid segments per round
    c_r = np.maximum(1, (N_r + 127) // 128).astype(np.int64)
    Qc = int(c_r.sum())
    off_r = np.concatenate([[0], np.cumsum(c_r)[:-1]]).astype(np.int64)

    # --- semi-dense MLP stream: round-major, no per-round 128-padding ---
    cum = np.concatenate([[0], np.cumsum(N_r)]).astype(np.int64)
    QD = int(cum[-1])
    NTILE_D = (QD + 511) // 512
    QD_pad = 512 * NTILE_D
    GPOS = 512 * GRP
    ngroups = (NTILE_D + GRP - 1) // GRP

    groups = []
    for g in range(ngroups):
        g0, g1 = GPOS * g, min(GPOS * (g + 1), QD)
        subdmas = []  # (slot, ncols, p0, p1, j0): ots2[comp, j0...] -> st rows
        cov = {}      # slot -> (p0, p1)
        slot0 = None
        for r in range(Vmax):
            dA, dB = max(int(cum[r]), g0), min(int(cum[r + 1]), g1)
            if dA >= dB:
                continue
            sA, sB = dA - int(cum[r]), dB - int(cum[r])
            a0, a1 = sA // 128, sB // 128
            pA, pB = sA % 128, sB % 128

            def slot_of(a, r=r):
                return int(off_r[r]) + a

            if slot0 is None:
                slot0 = slot_of(a0)
            j = dA - g0
            if a0 == a1:
                subdmas.append((slot_of(a0) - slot0, 1, pA, pB, j))
                cov[slot_of(a0)] = (pA, pB)
            else:
                if pA:
                    subdmas.append((slot_of(a0) - slot0, 1, pA, 128, j))
                    cov[slot_of(a0)] = (pA, 128)
                    j += 128 - pA
                    mstart = a0 + 1
                else:
                    mstart = a0
                if a1 > mstart:
                    subdmas.append((slot_of(mstart) - slot0,
                                    a1 - mstart, 0, 128, j))
                    for aa in range(mstart, a1):
                        cov[slot_of(aa)] = (0, 128)
                    j += 128 * (a1 - mstart)
                if pB:
                    subdmas.append((slot_of(a1) - slot0, 1, 0, pB, j))
                    cov[slot_of(a1)] = (0, pB)
        nslot = max(cov.keys()) - slot0 + 1
        groups.append(dict(g=g, tiles=(GRP * g, min(GRP * (g + 1), NTILE_D)),
                           slot0=slot0, nslot=nslot, subdmas=subdmas, cov=cov))

    # chunks: runs of rounds sharing the group in which their data completes
    thresh_r = [int(np.searchsorted(
        np.arange(1, ngroups + 1) * GPOS, int(cum[r + 1]) - 1, side="right"))
        for r in range(Vmax)]
    thresh_r = [min(t, ngroups - 1) for t in thresh_r]
    chunks = []
    start_r = 0
    for r in range(Vmax):
        if r + 1 == Vmax or thresh_r[r + 1] != thresh_r[r]:
            col0 = int(off_r[start_r])
            end_col = int(off_r[r] + c_r[r])
            chunks.append((start_r, r + 1, col0, end_col - col0,
                           thresh_r[r]))
            start_r = r + 1

    # per-group copy table: (chunk_index, local_col, slot_local, n, p0, p1)
    chunk_of_col = np.zeros(Qc, dtype=np.int64)
    for ci, (_, _, col0, w, _) in enumerate(chunks):
        chunk_of_col[col0:col0 + w] = ci
    for grp in groups:
        copies = []
        slots = sorted(grp["cov"].keys())
        i = 0
        while i < len(slots):
            col = slots[i]
            ci = int(chunk_of_col[col])
            p0, p1 = grp["cov"][col]
            n = 1
            while (i + n < len(slots) and slots[i + n] == col + n
                   and int(chunk_of_col[col + n]) == ci
                   and grp["cov"][col + n] == (p0, p1)):
                n += 1
            copies.append((ci, col - chunks[ci][2], col - grp["slot0"],
                           n, p0, p1))
            i += n
        grp["copies"] = copies

    return dict(Vmax=Vmax, c_r=c_r, off_r=off_r, Qc=Qc, Q=128 * Qc,
                seg_order=seg_order, chunks=chunks, N_r=N_r, cum=cum,
                QD=QD, QD_pad=QD_pad, NTILE_D=NTILE_D, groups=groups)


def _build_host_tensors(inputs, lay):
    kc = np.asarray(inputs["kc"]).astype(np.int64)
    corr = np.asarray(inputs["corr"]).astype(np.int64)
    FM = np.ascontiguousarray(np.asarray(inputs["FM"], dtype=np.float32))
    obs = np.asarray(inputs["obs_logits"], dtype=np.float32)
    trans = np.asarray(inputs["trans_logits"], dtype=np.float32)
    init = np.asarray(inputs["init_logits"], dtype=np.float32)

    Vmax, c_r, off_r, Qc, Q = (lay["Vmax"], lay["c_r"], lay["off_r"],
                               lay["Qc"], lay["Q"])
    FMf = FM.reshape(-1, NF)

    per_core = []
    for m in range(NCORES):
        seg = lay["seg_order"][m]
        seg_rank = np.empty(BPC * NK, dtype=np.int64)
        seg_rank[seg] = np.arange(BPC * NK)

        perm = np.zeros(Q, dtype=np.int64)
        valid = np.zeros(Q, dtype=bool)

        for bl in range(BPC):
            b = m * BPC + bl
            ord_t = np.argsort(kc[b], kind="stable")
            ch = kc[b][ord_t]
            visit = np.arange(T) - np.searchsorted(ch, ch)
            s = seg_rank[bl * NK + ch]
            q = (off_r[visit] + s // 128) * 128 + (s % 128)
            perm[q] = b * T + ord_t
            valid[q] = True

        rows = perm
        ch_of_q = kc.reshape(-1)[rows]
        y_of_q = corr.reshape(-1)[rows]

        def plane(vals):
            return np.ascontiguousarray(vals.reshape(Qc, 128).T)

        og = obs[ch_of_q].astype(np.float64)
        tg = trans[ch_of_q].astype(np.float64)
        # obs-logit difference (log-odds of obs=1) per state
        ogdv = og[:, :, 1] - og[:, :, 0]
        ogd = np.concatenate([plane(ogdv[:, 0].astype(np.float32)),
                              plane(ogdv[:, 1].astype(np.float32))], axis=1)
        # exact transition probs P(next=0 | prev=j) (softmax over dim 0)
        t0pv = 1.0 / (1.0 + np.exp(tg[:, 1, :] - tg[:, 0, :]))
        t0p = np.concatenate([plane(t0pv[:, 0].astype(np.float32)),
                              plane(t0pv[:, 1].astype(np.float32))], axis=1)
        t1p = np.concatenate([plane((1.0 - t0pv[:, 0]).astype(np.float32)),
                              plane((1.0 - t0pv[:, 1]).astype(np.float32))],
                             axis=1)
        sgn = plane((2.0 * y_of_q - 1.0).astype(np.float32))

        Sc = 32
        igf = np.zeros((128, 2 * Sc), dtype=np.float32)
        seg_chain = seg % NK
        sl = np.arange(BPC * NK)
        igf[sl % 128, sl // 128] = init[seg_chain, 0]
        igf[sl % 128, Sc + sl // 128] = init[seg_chain, 1]

        xT = np.ascontiguousarray(FMf[rows].T)
        if MM_BF16:
            from ml_dtypes import bfloat16
            xT = xT.astype(bfloat16)

        per_core.append(dict(
            xT=xT,
            ogd=np.ascontiguousarray(ogd, dtype=np.float32),
            t0p=np.ascontiguousarray(t0p, dtype=np.float32),
            t1p=np.ascontiguousarray(t1p, dtype=np.float32),
            sgn=np.ascontiguousarray(sgn, dtype=np.float32),
            ig=igf,
            perm=perm, valid=valid,
        ))

    w1 = np.ascontiguousarray(np.asarray(inputs["W1"], np.float32))
    b1r = np.ascontiguousarray(
        np.asarray(inputs["b1"], np.float32).reshape(4, 128).T)
    w2r = np.ascontiguousarray(
        np.asarray(inputs["W2"], np.float32).reshape(4, 128, 2)
        .transpose(1, 0, 2).reshape(128, 8))
    b2 = np.ascontiguousarray(np.asarray(inputs["b2"], np.float32))
    if MM_BF16:
        from ml_dtypes import bfloat16
        w1 = w1.astype(bfloat16)
        w2r = w2r.astype(bfloat16)
    shared = dict(w1=w1, b1r=b1r, w2r=w2r, b2=b2)
    return per_core, shared


# ---------------------------------------------------------------------------
# bass kernel
# ---------------------------------------------------------------------------

def _r2(ap, w2):
    """[128, 2*w] -> [128, 2, w] plane split."""
    return ap.rearrange("p (s w) -> p s w", s=2)


def _kernel_body(ctx, tc, lay, dram, repeat=1):
    nc = tc.nc
    Vmax, c_r, off_r, Qc, Q = (lay["Vmax"], lay["c_r"], lay["off_r"],
                               lay["Qc"], lay["Q"])
    NTILE = Q // 512
    cmax = int(max(c_r))

    singles = ctx.enter_context(tc.tile_pool(name="singles", bufs=1))
    xt_pool = ctx.enter_context(tc.tile_pool(name="xt", bufs=4))
    ht_pool = ctx.enter_context(tc.tile_pool(name="ht", bufs=3))
    sm_pool = ctx.enter_context(tc.tile_pool(name="sm", bufs=3))
    rpool = ctx.enter_context(tc.tile_pool(name="rounds", bufs=2))
    psum = ctx.enter_context(tc.tile_pool(name="psum", bufs=1, space="PSUM"))
    psum2 = ctx.enter_context(tc.tile_pool(name="psum2", bufs=2, space="PSUM"))

    ident = singles.tile([P, P], F32, tag="ident")
    make_identity(nc, ident)

    for _rep in range(repeat):
        _kernel_rep(tc, lay, dram, singles, xt_pool, ht_pool, sm_pool, rpool,
                    psum, psum2, ident)


def _kernel_rep(tc, lay, dram, singles, xt_pool, ht_pool, sm_pool, rpool,
                psum, psum2, ident):
    nc = tc.nc
    Vmax, c_r, off_r, Qc, Q = (lay["Vmax"], lay["c_r"], lay["off_r"],
                               lay["Qc"], lay["Q"])
    NTILE = Q // 512
    cmax = int(max(c_r))
    chunks = lay["chunks"]

    # --- small weights early on the ACT HWDGE ring; xt owns the SP ring ---
    MMDT = BF16 if MM_BF16 else F32R
    w1v = dram["w1"].rearrange("(k p) n -> p k n", p=P)
    w1sb = [singles.tile([P, 512], MMDT, tag=f"w1sb{k}", name=f"w1sb{k}")
            for k in range(4)]
    w2sb = singles.tile([P, 8], MMDT, tag="w2sb")
    for k in range(4):
        nc.scalar.dma_start(out=w1sb[k], in_=w1v[:, k, :])
    nc.scalar.dma_start(out=w2sb, in_=dram["w2r"])
    b1sb = singles.tile([P, 4], F32, tag="b1sb")
    nc.scalar.dma_start(out=b1sb, in_=dram["b1r"])
    b2sb = singles.tile([2, 1], F32, tag="b2sb")
    nc.scalar.dma_start(out=b2sb, in_=dram["b2"])

    ogdt = singles.tile([P, 2 * Qc], F32, tag="ogdt")
    t0pt = singles.tile([P, 2 * Qc], F32, tag="t0pt")
    t1pt = singles.tile([P, 2 * Qc], F32, tag="t1pt")
    sgnt = singles.tile([P, Qc], F32, tag="sgnt")
    igt = singles.tile([P, 64], F32, tag="igt")

    # pw planes: s0,s1 = alpha state (na), s2,s3 = py (linear space)
    pwt = singles.tile([P, 4 * Qc], F32, tag="pwt")
    pw5 = pwt.rearrange("p (s w) -> p s w", s=4)
    xTv = dram["xT"].rearrange("(k p) q -> p k q", p=P)

    ocat_ch = [singles.tile([P, 2 * w], F32, tag=f"ocat{ci}", name=f"ocat{ci}")
               for ci, (_, _, _, w) in enumerate(chunks)]
    kpl_ch = [singles.tile([P, 8 * w], F32, tag=f"kpl{ci}", name=f"kpl{ci}")
              for ci, (_, _, _, w) in enumerate(chunks)]
    chunk_of_col = np.zeros(Qc, dtype=np.int64)
    for ci, (_, _, col0, w) in enumerate(chunks):
        chunk_of_col[col0:col0 + w] = ci

    state = dict(prev=None)  # [P, 2, w] AP of the previous round's alpha

    def emit_plane_loads():
        nc.gpsimd.dma_start(out=ogdt, in_=dram["ogd"])
        nc.gpsimd.dma_start(out=t0pt, in_=dram["t0p"])
        nc.gpsimd.dma_start(out=t1pt, in_=dram["t1p"])
        nc.gpsimd.dma_start(out=sgnt, in_=dram["sgn"])
        nc.gpsimd.dma_start(out=igt, in_=dram["ig"])
        # init state: a1 = sigmoid(ig1-ig0) = 0.5 + 0.5*tanh((ig1-ig0)/2)
        ad = sm_pool.tile([P, 32], F32, tag="ad", name="ad")
        nc.vector.tensor_sub(ad, igt[:, 32:64], igt[:, 0:32])
        th = sm_pool.tile([P, 32], F32, tag="th", name="th")
        nc.scalar.activation(out=th, in_=ad, func=AF.Tanh, scale=0.5)
        vinit = singles.tile([P, 64], F32, tag="vinit")
        nc.vector.tensor_scalar(out=vinit[:, 32:64], in0=th,
                                scalar1=0.5, scalar2=0.5,
                                op0=OP.mult, op1=OP.add)
        nc.vector.tensor_scalar(out=vinit[:, 0:32], in0=th,
                                scalar1=-0.5, scalar2=0.5,
                                op0=OP.mult, op1=OP.add)
        state["prev"] = vinit.rearrange("p (j w) -> p j w", j=2)

    def phase_b_and_rounds(ci):
        r0, r1, col0, w = chunks[ci]
        oc = ocat_ch[ci]
        o2c = sm_pool.tile([P, 2 * cmax], F32, tag="o2c",
                           name=f"o2c{ci}")[:, 0:2 * w]
        nc.vector.tensor_scalar_mul(o2c, oc, 2.0)
        g = sm_pool.tile([P, 4 * cmax], F32, tag="g", name=f"g{ci}")[:, 0:4 * w]
        sg = sm_pool.tile([P, 4 * cmax], F32, tag="sg",
                          name=f"sg{ci}")[:, 0:4 * w]
        # g = [pe-logit (2w) | obs1-logit (2w)]
        nc.vector.tensor_tensor(out=_r2(g[:, 2 * w:4 * w], w),
                                in0=_r2(ogdt, Qc)[:, :, col0:col0 + w],
                                in1=_r2(o2c, w), op=OP.subtract)
        nc.vector.tensor_tensor(
            out=_r2(g[:, 0:2 * w], w), in0=_r2(g[:, 2 * w:4 * w], w),
            in1=sgnt[:, col0:col0 + w].unsqueeze(1).broadcast_to([P, 2, w]),
            op=OP.mult)
        # sigmoid(x) = 0.5 + 0.5*tanh(x/2): keep ACT on the tanh table set
        nc.scalar.activation(out=sg, in_=g, func=AF.Tanh, scale=0.5)
        nc.vector.tensor_scalar(out=sg, in0=sg, scalar1=0.5, scalar2=0.5,
                                op0=OP.mult, op1=OP.add)
        # sg = [pe0,pe1 | p01,p11] (probabilities); T planes precomputed on host
        kt = kpl_ch[ci]
        k4 = kt.rearrange("p (h q w) -> p h q w", h=2, q=4)
        nc.vector.tensor_scalar(out=k4[:, :, 2, :], in0=_r2(sg[:, 2 * w:4 * w], w),
                                scalar1=-1.0, scalar2=1.0,
                                op0=OP.mult, op1=OP.add)
        nc.vector.tensor_copy(out=k4[:, :, 3, :], in_=_r2(sg[:, 2 * w:4 * w], w))
        nc.vector.tensor_tensor(out=k4[:, :, 0, :],
                                in0=_r2(t0pt, Qc)[:, :, col0:col0 + w],
                                in1=_r2(sg[:, 0:2 * w], w), op=OP.mult)
        nc.vector.tensor_tensor(out=k4[:, :, 1, :],
                                in0=_r2(t1pt, Qc)[:, :, col0:col0 + w],
                                in1=_r2(sg[:, 0:2 * w], w), op=OP.mult)

        k4v = kt.rearrange("p (j q w) -> p j q w", j=2, q=4)
        for r in range(r0, r1):
            c = int(c_r[r]); off = int(off_r[r]); offl = off - col0
            prev = state["prev"]
            u = rpool.tile([P, 8 * cmax], F32, tag="u", name=f"u{r}")[:, 0:8 * c]
            src = prev[:, :, 0:c].unsqueeze(2).broadcast_to([P, 2, 4, c])
            nc.vector.tensor_tensor(
                out=u.rearrange("p (j q w) -> p j q w", j=2, q=4),
                in0=src, in1=k4v[:, :, :, offl:offl + c], op=OP.mult)
            # one add produces [na0 na1 py0 py1] for this round's columns
            nc.vector.tensor_tensor(
                out=pw5[:, 0:4, off:off + c],
                in0=u[:, 0:4 * c].rearrange("p (q w) -> p q w", q=4),
                in1=u[:, 4 * c:8 * c].rearrange("p (q w) -> p q w", q=4),
                op=OP.add)
            if c > 2:
                nc.vector.tensor_scalar_max(pw5[:, 0:2, off:off + c],
                                            pw5[:, 0:2, off:off + c], 1e-20)
            state["prev"] = pw5[:, 0:2, off:off + c]

    next_chunk = [0]
    GRP = 4  # tiles per batched PE transpose

    def finish_group(g, st32, ntl):
        # one transpose covers ntl tiles (8 rows each)
        pt = psum2.tile([P, 8 * GRP], F32, tag="pt", name=f"pt{g}")[:, 0:8 * ntl]
        nc.tensor.transpose(out=pt, in_=st32[0:8 * ntl, :],
                            identity=ident[0:8 * ntl, 0:8 * ntl])
        for tl in range(ntl):
            n = g * GRP + tl
            ci = int(chunk_of_col[4 * n])
            _, _, col0, w = chunks[ci]
            nc.vector.tensor_copy(
                out=_r2(ocat_ch[ci], w)[:, :, 4 * n - col0:4 * n - col0 + 4],
                in_=pt[:, 8 * tl:8 * tl + 8].rearrange("p (s c) -> p s c", s=2))
        n_done = g * GRP + ntl
        while (next_chunk[0] < len(chunks)
               and chunks[next_chunk[0]][2] + chunks[next_chunk[0]][3]
               <= 4 * n_done):
            phase_b_and_rounds(next_chunk[0])
            next_chunk[0] += 1

    grp_q = []
    st32 = [None]
    ht_q = []

    def emit_w2(n, ht):
        # W2 runs one tile behind W1 so its ACT-wait never stalls the PE FIFO
        po = psum2.tile([2, 512], F32, tag="po", name=f"po{n}")
        for k in range(4):
            nc.tensor.matmul(po, lhsT=w2sb[:, 2 * k:2 * k + 2],
                             rhs=ht[:, k, :], start=(k == 0), stop=(k == 3))
        ots = sm_pool.tile([2, 512], F32, tag="ots", name=f"ots{n}")
        nc.vector.tensor_scalar(out=ots, in0=po, scalar1=b2sb, scalar2=None,
                                op0=OP.add)
        tl = n % GRP
        if tl == 0:
            st32[0] = sm_pool.tile([8 * GRP, 128], F32, tag="st32",
                                   name=f"st32_{n // GRP}")
        nc.gpsimd.dma_start(out=st32[0][8 * tl:8 * tl + 8, :],
                            in_=ots.rearrange("s (c x) -> s c x", c=4))
        if tl == GRP - 1 or n == NTILE - 1:
            grp_q.append((n // GRP, st32[0], tl + 1))
            if len(grp_q) >= 2:
                finish_group(*grp_q.pop(0))

    for n in range(NTILE):
        if n == 2:
            emit_plane_loads()
        xdma = nc.sync.dma_start
        if n == 0:
            xt0 = [xt_pool.tile([P, 512], MMDT, tag=f"xt0_{k}",
                                name=f"xt0_{k}") for k in range(4)]
            for k in range(4):
                xdma(out=xt0[k], in_=xTv[:, k, 0:512])
            xtk = lambda k: xt0[k]
        else:
            xt = xt_pool.tile([P, 4, 512], MMDT, tag="xt", name=f"xt{n}")
            xdma(out=xt, in_=xTv[:, :, n * 512:(n + 1) * 512])
            xtk = lambda k: xt[:, k, :]
        ht = ht_pool.tile([P, 4, 512], MMDT, tag="ht", name=f"ht{n}")
        for m in range(4):
            ph = psum.tile([P, 512], F32, tag=f"h{m}", name=f"h{m}_{n}")
            for k in range(4):
                nc.tensor.matmul(
                    ph,
                    lhsT=w1sb[k][:, m * 128:(m + 1) * 128],
                    rhs=xtk(k),
                    start=(k == 0), stop=(k == 3))
            nc.scalar.activation(out=ht[:, m, :], in_=ph, func=AF.Tanh,
                                 bias=b1sb[:, m:m + 1], scale=1.0)
        ht_q.append((n, ht))
        if len(ht_q) >= 2:
            emit_w2(*ht_q.pop(0))

    while ht_q:
        emit_w2(*ht_q.pop(0))
    while grp_q:
        finish_group(*grp_q.pop(0))
    while next_chunk[0] < len(chunks):
        phase_b_and_rounds(next_chunk[0])
        next_chunk[0] += 1

    # ship linear-space py planes; the host does log(py_s) - log(py0+py1)
    nc.gpsimd.dma_start(out=dram["out"], in_=pwt[:, 2 * Qc:4 * Qc])


def _build_nc(lay, repeat=1):
    from contextlib import ExitStack
    nc = bacc.Bacc("TRN2", target_bir_lowering=False, debug=False,
                   num_devices=NCORES)
    Qc, Q = lay["Qc"], lay["Q"]
    dram = {}
    def din(name, shape, dt=F32):
        dram[name] = nc.dram_tensor(name, shape, dt, kind="ExternalInput").ap()
    mmin = BF16 if MM_BF16 else F32R
    din("xT", [NF, Q], mmin)
    din("w1", [NF, NH], mmin)
    din("b1r", [P, 4])
    din("w2r", [P, 8], mmin)
    din("b2", [2])
    din("ogd", [P, 2 * Qc])
    din("t0p", [P, 2 * Qc])
    din("t1p", [P, 2 * Qc])
    din("sgn", [P, Qc])
    din("ig", [P, 64])
    dram["out"] = nc.dram_tensor("out", [P, 2 * Qc], F32,
                                 kind="ExternalOutput").ap()
    with tile.TileContext(nc) as tc:
        with ExitStack() as ctx:
            _kernel_body(ctx, tc, lay, dram, repeat=repeat)
    nc.compile()
    return nc


_NC_CACHE = {}


def _get_nc(lay):
    key = tuple(int(x) for x in lay["c_r"])
    if key not in _NC_CACHE:
        _NC_CACHE[key] = _build_nc(lay)
    return _NC_CACHE[key]


# ---------------------------------------------------------------------------
# entry point
# ---------------------------------------------------------------------------

def kernel(corr, kc, FM, W1, b1, W2, b2, trans_logits, obs_logits, init_logits,
           _want_results_only=True, _trace=False):
    inputs = dict(corr=corr, kc=kc, FM=FM, W1=W1, b1=b1, W2=W2, b2=b2,
                  trans_logits=trans_logits, obs_logits=obs_logits,
                  init_logits=init_logits)
    lay = _build_layout(kc)
    nc = _get_nc(lay)
    per_core, shared = _build_host_tensors(inputs, lay)

    in_maps = []
    for m in range(NCORES):
        c = per_core[m]
        in_maps.append(dict(
            xT=c["xT"], w1=shared["w1"], b1r=shared["b1r"], w2r=shared["w2r"],
            b2=shared["b2"], ogd=c["ogd"], t0p=c["t0p"], t1p=c["t1p"],
            sgn=c["sgn"], ig=c["ig"]))

    res = run_bass_kernel_spmd(nc, in_maps, core_ids=list(range(NCORES)),
                               trace=_trace)

    out = np.zeros((B * T, 2), dtype=np.float32)
    for m in range(NCORES):
        scatter_core_output(lay, per_core[m], res.results[m]["out"], out)
    out = out.reshape(B, T, 2)
    if _want_results_only:
        return out
    return out, res


def scatter_core_output(lay, core, OUT, out):
    """OUT: [P, 2*Qc] linear-space py planes -> log-prob rows of `out`."""
    Qc, Q = lay["Qc"], lay["Q"]
    J = np.arange(Q) // 128
    p = np.arange(Q) % 128
    g = core["perm"]; v = core["valid"]
    py0 = OUT[p[v], J[v]].astype(np.float64)
    py1 = OUT[p[v], Qc + J[v]].astype(np.float64)
    s = np.log(py0 + py1)
    out[g[v], 0] = np.log(py0) - s
    out[g[v], 1] = np.log(py1) - s

